# revision 7
# baseline (speedup 1.0000x reference)
"""MoE layer (top-2 routing, SwiGLU experts) for Trainium2, 8 NeuronCores.

Strategy: expert parallelism — one expert per core. The gate (0.03% of
FLOPs) and the token dispatch/combine run on host; each core runs the
dense SwiGLU FFN for the tokens routed to its expert in bf16 (PSUM
accumulation is fp32; tolerance is 2e-2, bf16 lands ~4e-3):

  phase 1:  h = silu(x @ w1) * (x @ w3)      (spilled to DRAM, [I, C] blocked)
  phase 2:  y = (h @ w2) * route_weight      (tokens on partitions)

Tokens are gathered per expert on host, padded to a common capacity C
(multiple of 128), and x is passed transposed ([H, C]) so both matmul
phases stream with tokens on the free dimension (phase 1) / stationary
dimension (phase 2) without any on-device transposes. w2 is prefetched
during phase 1 so the phase transition doesn't stall on its 8MB load.
"""

import os
import sys

for _p in ("/opt/trn_rl_repo", "/root/.axon_site/_ro/trn_rl_repo"):
    if os.path.isdir(_p) and _p not in sys.path:
        sys.path.insert(0, _p)

import numpy as np

import concourse.bass as bass  # noqa: F401  (bass must import before bacc)
import concourse.mybir as mybir
import concourse.tile as tile
from concourse import bacc
from concourse.bass_utils import run_bass_kernel_spmd

H = 1024
E = 8
I = 4096
TOP_K = 2
P = 128
NTOK = 512  # token tile width in phase 1 (PSUM bank = 512 fp32)
F32 = mybir.dt.float32
AF = mybir.ActivationFunctionType

_programs: dict = {}


def build_program(C: int, repeat: int = 1, dtype: str = "bf16", tg_size: int = 3,
                  ph: str = "both", silu_af: bool = True) -> "bacc.Bacc":
    """One-core SPMD program: SwiGLU FFN for C tokens of one expert.

    repeat > 1 re-emits the whole computation (benchmarking aid: the HW
    time difference between repeat=1+R and repeat=1 is R clean iterations).
    dtype: matmul operand precision — "f32" (4-pass, exact), "f32r"
    (FP22, 1-pass at >=256 moving width), or "bf16". PSUM is fp32 always.
    """
    assert C % P == 0
    DT = {"f32": F32, "f32r": mybir.dt.float32r, "bf16": mybir.dt.bfloat16}[dtype]
    Cb = C // P
    HB = H // P  # 8
    IB = I // P  # 32
    NH = H // NTOK  # 2
    # phase-1 token tile widths (512s + one 128-multiple remainder)
    tts = [NTOK] * (C // NTOK)
    if C % NTOK:
        tts.append(C % NTOK)
    starts = np.cumsum([0] + tts[:-1]).tolist()
    groups = [list(range(g, min(g + tg_size, len(tts))))
              for g in range(0, len(tts), tg_size)]

    nc = bacc.Bacc("TRN2", target_bir_lowering=False, debug=False, num_devices=8)
    x_d = nc.dram_tensor("xt", [H, C], DT, kind="ExternalInput")
    w1_d = nc.dram_tensor("w1", [H, I], DT, kind="ExternalInput")
    w3_d = nc.dram_tensor("w3", [H, I], DT, kind="ExternalInput")
    w2_d = nc.dram_tensor("w2", [I, H], DT, kind="ExternalInput")
    s_d = nc.dram_tensor("st", [P, Cb], F32, kind="ExternalInput")
    # one output region per repeat so no iteration is dead code
    y_d = nc.dram_tensor("y", [repeat * C, H], F32, kind="ExternalOutput")
    # h_act scratch, blocked [c-block, i-block, i-sub(part), c-sub] so that
    # phase-1 writes and phase-2 reads are both contiguous
    ha_d = nc.dram_tensor("hact", [Cb, IB, P, P], DT)

    xr = x_d.rearrange("(h p) c -> p h c", p=P)  # [128, 8, C]
    w1r = w1_d.rearrange("(h p) i -> p h i", p=P)  # [128, 8, 4096]
    w3r = w3_d.rearrange("(h p) i -> p h i", p=P)
    w2r = w2_d.rearrange("(i p) n -> p i n", p=P)  # [128, 32, 1024]

    with tile.TileContext(nc) as tc:
      for rep in range(repeat):
        with (
            tc.tile_pool(name=f"xt{rep}", bufs=1) as xt_pool,
            tc.tile_pool(name=f"w13{rep}", bufs=4) as w13_pool,
            tc.tile_pool(name=f"tmp{rep}", bufs=4) as tmp_pool,
            tc.tile_pool(name=f"hst{rep}", bufs=4) as hst_pool,
            tc.tile_pool(name=f"ps1{rep}", bufs=2 * 3, space="PSUM") as ps_pool,
            tc.tile_pool(name=f"w2{rep}", bufs=1) as w2_pool,
            tc.tile_pool(name=f"sc{rep}", bufs=1) as s_pool,
            tc.tile_pool(name=f"hld{rep}", bufs=3) as hld_pool,
            tc.tile_pool(name=f"ysb{rep}", bufs=3) as y_pool,
        ):
            yps_pool = ps_pool  # phase 2 reuses the phase-1 PSUM banks
            # ---------------- phase 1: h = silu(x@w1) * (x@w3) ----------------
            w2ts = []
            if ph in ("both", "p1"):
                # issue order: w13[i=0], x tiles of group 0 — the minimal set
                # for the first matmuls — then the rest of x; w2 prefetch is
                # interleaved into the i-loop so it never starves w13 loads
                w13ts = {}
                w1t = w13_pool.tile([P, HB, P], DT, tag="w13", name="w1_0")
                w3t = w13_pool.tile([P, HB, P], DT, tag="w13", name="w3_0")
                nc.sync.dma_start(out=w1t[:], in_=w1r[:, :, 0:P])
                nc.sync.dma_start(out=w3t[:], in_=w3r[:, :, 0:P])
                w13ts[0] = (w1t, w3t)
                xts = [[None] * HB for _ in tts]
                for tg in groups:
                    for t in tg:
                        for h in range(HB):
                            xtile = xt_pool.tile([P, tts[t]], DT, tag=f"x{h}_{t}",
                                                 name=f"x{h}_{t}")
                            nc.sync.dma_start(
                                out=xtile[:],
                                in_=xr[:, h, starts[t]: starts[t] + tts[t]])
                            xts[t][h] = xtile
                for i in range(1, min(4, IB)):
                    w1t = w13_pool.tile([P, HB, P], DT, tag="w13", name=f"w1_{i}")
                    w3t = w13_pool.tile([P, HB, P], DT, tag="w13", name=f"w3_{i}")
                    nc.sync.dma_start(out=w1t[:], in_=w1r[:, :, i * P: (i + 1) * P])
                    nc.sync.dma_start(out=w3t[:], in_=w3r[:, :, i * P: (i + 1) * P])
                    w13ts[i] = (w1t, w3t)
                if ph == "both":
                    st = s_pool.tile([P, Cb], F32, tag="st", name="st")
                    nc.sync.dma_start(out=st[:], in_=s_d[:])

                for i in range(IB):
                    if i in w13ts:
                        w1t, w3t = w13ts[i]
                    else:
                        w1t = w13_pool.tile([P, HB, P], DT, tag="w13", name=f"w1_{i}")
                        w3t = w13_pool.tile([P, HB, P], DT, tag="w13", name=f"w3_{i}")
                        nc.sync.dma_start(out=w1t[:], in_=w1r[:, :, i * P: (i + 1) * P])
                        nc.sync.dma_start(out=w3t[:], in_=w3r[:, :, i * P: (i + 1) * P])
                    # phase-2 weight prefetch, 2 tiles per iteration
                    if ph == "both" and i < IB // 2:
                        for j in (2 * i, 2 * i + 1):
                            w2t = w2_pool.tile([P, H], DT, tag=f"w2_{j}", name=f"w2_{j}")
                            nc.sync.dma_start(out=w2t[:], in_=w2r[:, j, :])
                            w2ts.append(w2t)
                    # token-tile groups: each stationary weight serves the whole
                    # group before switching (amortizes LDWEIGHTS)
                    for tg in groups:
                        p1s, p3s = {}, {}
                        for t in tg:
                            p1s[t] = ps_pool.tile([P, NTOK], F32, tag="ps", name=f"p1_{i}_{t}")
                            p3s[t] = ps_pool.tile([P, NTOK], F32, tag="ps", name=f"p3_{i}_{t}")
                        for h in range(HB):
                            for t in tg:
                                nc.tensor.matmul(
                                    p1s[t][:, : tts[t]], w1t[:, h, :], xts[t][h][:],
                                    start=(h == 0), stop=(h == HB - 1),
                                )
                        for h in range(HB):
                            for t in tg:
                                nc.tensor.matmul(
                                    p3s[t][:, : tts[t]], w3t[:, h, :], xts[t][h][:],
                                    start=(h == 0), stop=(h == HB - 1),
                                )
                        for t in tg:
                            w, c0, p1, p3 = tts[t], starts[t], p1s[t], p3s[t]
                            hst = hst_pool.tile([P, NTOK], DT, tag="hst", name=f"h_{i}_{t}")
                            tmp = tmp_pool.tile([P, NTOK], F32, tag="tmp", name=f"tmp_{i}_{t}")
                            if silu_af:
                                nc.scalar.activation(tmp[:, :w], p1[:, :w], AF.Silu)
                                nc.vector.tensor_mul(hst[:, :w], tmp[:, :w], p3[:, :w])
                            else:
                                # silu(p1)*p3 = sigmoid(p1)*p1*p3
                                t2 = tmp_pool.tile([P, NTOK], F32, tag="tmp2", name=f"t2_{i}_{t}")
                                nc.scalar.activation(tmp[:, :w], p1[:, :w], AF.Sigmoid)
                                nc.vector.tensor_mul(t2[:, :w], tmp[:, :w], p1[:, :w])
                                nc.vector.tensor_mul(hst[:, :w], t2[:, :w], p3[:, :w])
                            for k in range(w // P):
                                cb = c0 // P + k
                                nc.sync.dma_start(
                                    out=ha_d[cb, i], in_=hst[:, k * P: (k + 1) * P]
                                )

            # ---------------- phase 2: y = (h @ w2) * s ----------------
            if ph in ("both", "p2"):
                if ph == "p2":
                    st = s_pool.tile([P, Cb], F32, tag="st", name="st")
                    nc.sync.dma_start(out=st[:], in_=s_d[:])
                    for i in range(IB):
                        w2t = w2_pool.tile([P, H], DT, tag=f"w2_{i}", name=f"w2_{i}")
                        nc.sync.dma_start(out=w2t[:], in_=w2r[:, i, :])
                        w2ts.append(w2t)
                for cb in range(Cb):
                    hld = hld_pool.tile([P, IB, P], DT, tag="hld", name=f"hld_{cb}")
                    nc.sync.dma_start(out=hld[:], in_=ha_d[cb].rearrange("i p c -> p i c"))
                    # n-inner: each hld stationary serves both n halves
                    yps = [
                        yps_pool.tile([P, NTOK], F32, tag="ps", name=f"yp_{cb}_{n}")
                        for n in range(NH)
                    ]
                    for i in range(IB):
                        for n in range(NH):
                            nc.tensor.matmul(
                                yps[n][:],
                                hld[:, i, :],
                                w2ts[i][:, n * NTOK: (n + 1) * NTOK],
                                start=(i == 0), stop=(i == IB - 1),
                            )
                    for n in range(NH):
                        ysb = y_pool.tile([P, NTOK], F32, tag="ysb", name=f"y_{cb}_{n}")
                        nc.scalar.activation(
                            ysb[:], yps[n][:], AF.Copy, scale=st[:, cb: cb + 1]
                        )
                        nc.sync.dma_start(
                            out=y_d[
                                rep * C + cb * P: rep * C + (cb + 1) * P,
                                n * NTOK: (n + 1) * NTOK,
                            ],
                            in_=ysb[:],
                        )

    nc.compile()
    return nc


DTYPE = os.environ.get("MOE_DTYPE", "bf16")


def get_program(C: int) -> "bacc.Bacc":
    key = (C, DTYPE)
    if key not in _programs:
        _programs[key] = build_program(C, dtype=DTYPE)
    return _programs[key]


def _gate(x: np.ndarray, gate_w: np.ndarray):
    """Top-2 routing, mirroring the jax reference (softmax -> top_k ->
    renormalize). Uses jax for bit-compatible selection when available."""
    try:
        import jax
        import jax.numpy as jnp

        logits = jnp.asarray(x) @ jnp.asarray(gate_w)
        probs = jax.nn.softmax(logits, axis=-1)
        top_vals, top_idx = jax.lax.top_k(probs, TOP_K)
        top_vals = top_vals / jnp.sum(top_vals, axis=-1, keepdims=True)
        return np.asarray(top_vals), np.asarray(top_idx)
    except Exception:
        logits = x @ gate_w
        m = logits.max(-1, keepdims=True)
        p = np.exp(logits - m)
        p /= p.sum(-1, keepdims=True)
        top_idx = np.argsort(-p, axis=-1, kind="stable")[:, :TOP_K]
        top_vals = np.take_along_axis(p, top_idx, axis=-1)
        top_vals = top_vals / top_vals.sum(-1, keepdims=True)
        return top_vals, top_idx


def prepare_dispatch(x, gate_w):
    """Route tokens: per-expert index lists, routing weights, capacity C."""
    top_vals, top_idx = _gate(x, gate_w)
    idxs, wts = [], []
    for e in range(E):
        sel = top_idx == e  # [T, K] bool
        mask = sel.any(axis=-1)
        idx_e = np.nonzero(mask)[0]
        w_e = np.where(sel[idx_e, 0], top_vals[idx_e, 0], top_vals[idx_e, 1])
        idxs.append(idx_e)
        wts.append(w_e.astype(np.float32))
    max_cnt = max(len(ix) for ix in idxs)
    C = max(NTOK, -(-max_cnt // P) * P)
    return idxs, wts, C


def make_in_maps(x, w1, w3, w2, idxs, wts, C, dtype=None):
    dtype = dtype or DTYPE
    if dtype == "bf16":
        import ml_dtypes
        npdt = ml_dtypes.bfloat16
    else:
        npdt = np.float32
    Cb = C // P
    in_maps = []
    for e in range(E):
        cnt = len(idxs[e])
        x_pad = np.zeros((C, H), np.float32)
        x_pad[:cnt] = x[idxs[e]]
        s_pad = np.zeros(C, np.float32)
        s_pad[:cnt] = wts[e]
        in_maps.append(
            {
                "xt": np.ascontiguousarray(x_pad.T).astype(npdt),
                "w1": np.ascontiguousarray(np.asarray(w1[e], dtype=np.float32)).astype(npdt),
                "w3": np.ascontiguousarray(np.asarray(w3[e], dtype=np.float32)).astype(npdt),
                "w2": np.ascontiguousarray(np.asarray(w2[e], dtype=np.float32)).astype(npdt),
                "st": np.ascontiguousarray(s_pad.reshape(Cb, P).T),
            }
        )
    return in_maps


def combine(results, idxs, T):
    out = np.zeros((T, H), np.float32)
    for e in range(E):
        cnt = len(idxs[e])
        out[idxs[e]] += results[e]["y"][:cnt]
    return out


def kernel(hidden_states, gate_w, w1, w3, w2):
    B, S, Hh = hidden_states.shape
    assert Hh == H
    x = np.ascontiguousarray(hidden_states.reshape(-1, H), dtype=np.float32)
    T = x.shape[0]

    idxs, wts, C = prepare_dispatch(x, gate_w)
    nc = get_program(C)
    in_maps = make_in_maps(x, w1, w3, w2, idxs, wts, C)
    res = run_bass_kernel_spmd(nc, in_maps, list(range(E)))
    out = combine(res.results, idxs, T)
    return out.reshape(B, S, H)
